# revision 16
# baseline (speedup 1.0000x reference)
"""Weighted-MSE loss kernel (nn_LossWithEuler) for 8 Trainium2 NeuronCores.

loss = mean(weight[b] * (inp[d,b] - label[d,b])^2)
  weight[b]  = attr_w[b] * angle_w[b]
  attr_w[b]  = sum_j (attribute[j,b]==1) * (sum(attribute_num)/attribute_num[j])
  angle_w[b] = sum_j (1 - cos(ea[j,b]))

Sharding: batch axis B=131072 split across 8 cores (16384 each). Each core's
shard is host-transposed to (16384, 136) bf16 so that b sits on SBUF
partitions. inp/label travel as bf16 (the 2e-2 rel-err budget is ~500x above
the measured 4e-5 bf16 pipeline error).

Engine split (measured costs per 1088-elem chunk):
  DVE  subtract in place (0.71us, bf16 2x_1P) + the weight precompute
  ACT  square in place (1.2us)
  PE   per-b matmul with the WEIGHT as the stationary operand:
         psum[1, 0:136] += sum_p w[p,b] * sq[p, b, d]
       which fuses the weight multiply, the d-reduce and the partition
       reduce into one accumulating PSUM bank across all 128 b-matmuls
  -> every engine sits under the ~1.4us/chunk HBM stream pace.

The data stream runs on the qSP HWDGE queue in 1.11 MB DMAs (4352-byte
descriptors sustain ~410 GB/s; 2176-byte ones only ~270). Weight inputs
(anum/ea/attr-as-f32) are host-packed into one [128, 774] f32 tensor (one
3 KB descriptor per partition) on the qAct queue. The output is the single
accumulated PSUM bank reduced to one scalar, so the final DMA is one 4-byte
descriptor (a [128,1] store would be 128 4-byte HBM RMW writes, ~8us).
"""

import sys
import numpy as np

D = 136
B = 131072
N_CORES = 8
BS = B // N_CORES  # 16384 b's per core
P = 128            # SBUF partitions
Q = BS // P        # 128 b's per partition
NCHUNK = 16        # compute chunks over the free dim
CB = Q // NCHUNK   # b's per chunk
CF = CB * D        # free elements per chunk per tensor
DCH = 2            # compute chunks per DMA
NDMA = NCHUNK // DCH
PK = 6 + 3 * Q + 6 * Q  # packed weight-input words per partition

_program = None


def _build_program():
    try:
        import concourse.bass as bass
    except ImportError:
        sys.path.insert(0, "/opt/trn_rl_repo")
        import concourse.bass as bass
    from concourse import bacc, mybir, tile

    f32 = mybir.dt.float32
    bf16 = mybir.dt.bfloat16
    AF = mybir.ActivationFunctionType
    OP = mybir.AluOpType
    AX = mybir.AxisListType

    nc = bacc.Bacc("TRN2", target_bir_lowering=False, debug=False,
                   num_devices=N_CORES)

    # inp and label shards stacked on the host: data[0]=inp.T, data[1]=label.T
    data = nc.dram_tensor("data", (2, BS, D), bf16, kind="ExternalInput")
    # packed weight inputs: per partition [anum(6) | ea j-major(384) |
    # attr-as-f32 j-major(384)]
    pk = nc.dram_tensor("pk", (P, PK), f32, kind="ExternalInput")
    out = nc.dram_tensor("out", (1, 1), f32, kind="ExternalOutput")

    data_v = data.ap().rearrange("t (p q) d -> p t (q d)", p=P)

    with tile.TileContext(nc) as tc:
        with tc.tile_pool(name="const", bufs=1) as cpool, \
             tc.tile_pool(name="main", bufs=8) as mpool, \
             tc.tile_pool(name="psum", bufs=1, space="PSUM") as ppool:

            dts = {}

            def cslice(c):
                t, k, n = dts[c]
                return t[:, k * CF:(k + 1) * CF]

            def lslice(c):
                t, k, n = dts[c]
                return t[:, n * CF + k * CF:n * CF + (k + 1) * CF]

            # chunks per DMA group: uniform pairs, tapered tail so the last
            # chunks land (and finish computing) as early as possible.
            GROUPS = [2, 2, 2, 2, 2, 2, 2, 2]
            GOFF = [sum(GROUPS[:i]) for i in range(len(GROUPS))]

            def dma_group(g):
                n = GROUPS[g]
                t = mpool.tile([P, 2 * n * CF], bf16, tag="data")
                nc.sync.dma_start(
                    t[:].rearrange("p (t f) -> p t f", t=2),
                    data_v[:, :, GOFF[g] * CF:(GOFF[g] + n) * CF],
                )
                for k in range(n):
                    dts[GOFF[g] + k] = (t, k, n)

            dma_group(0)
            dma_group(1)

            # ---- packed weight-input DMA on the qAct HWDGE queue (any
            # SWDGE traffic measurably degrades the main qSP stream rate).
            pk_sb = cpool.tile([P, PK], f32)
            nc.scalar.dma_start(pk_sb[:], pk.ap())
            a_sb = pk_sb[:, 0:6]
            ea_sb = pk_sb[:, 6:6 + 3 * Q]
            attr_f = pk_sb[:, 6 + 3 * Q:PK]

            # ---- weight computation: runs in the chunk-0/1 DMA shadow.
            # angle_w first so ACT's sin (and its table load) happen early.
            sinh_sb = cpool.tile([P, 3 * Q], f32)
            nc.scalar.activation(sinh_sb[:], ea_sb, AF.Sin, bias=0.0,
                                 scale=0.5)
            ssq = cpool.tile([P, 3 * Q], f32)
            nc.vector.tensor_mul(ssq[:], sinh_sb[:], sinh_sb[:])
            angle = cpool.tile([P, Q], f32)
            nc.vector.tensor_add(angle[:], ssq[:, 0:Q], ssq[:, Q:2 * Q])
            nc.vector.tensor_add(angle[:], angle[:], ssq[:, 2 * Q:3 * Q])
            # inverse-frequency: ivb[p,j] = sum(anum)/anum[j]
            tot = cpool.tile([P, 1], f32)
            nc.vector.tensor_reduce(tot[:], a_sb, axis=AX.X, op=OP.add)
            rec = cpool.tile([P, 6], f32)
            nc.vector.reciprocal(rec[:], a_sb)
            ivb = cpool.tile([P, 6], f32)
            nc.vector.tensor_scalar_mul(ivb[:], rec[:], tot[:, 0:1])
            # attr_w[p,q] = sum_j attr[j, p*128+q] * iv[j]
            aw0 = cpool.tile([P, Q], f32)
            aw1 = cpool.tile([P, Q], f32)
            nc.vector.tensor_scalar_mul(aw0[:], attr_f[:, 0:Q], ivb[:, 0:1])
            cur, nxt = aw0, aw1
            for j in range(1, 6):
                nc.vector.scalar_tensor_tensor(
                    nxt[:], attr_f[:, j * Q:(j + 1) * Q], ivb[:, j:j + 1],
                    cur[:], op0=OP.mult, op1=OP.add,
                )
                cur, nxt = nxt, cur
            aw = cur
            # weight[p,q] = (2*angle) * attr_w, cast to bf16 for the PE
            # stationary operand (0.4% per-b rounding averages out over B).
            w_sb = cpool.tile([P, Q], f32)
            nc.vector.scalar_tensor_tensor(
                w_sb[:], angle[:], 2.0, aw[:], op0=OP.mult, op1=OP.mult,
            )
            w_bf = cpool.tile([P, Q], bf16)
            nc.vector.tensor_copy(w_bf[:], w_sb[:])

            # ---- accumulating PSUM bank for the whole weighted sum.
            acc = ppool.tile([P, 512], f32)
            mm_state = {"first": True}

            def mm_chunk(c, last=False):
                s = cslice(c)
                for b in range(CB):
                    nc.tensor.matmul(
                        acc[:1, 0:D],
                        w_bf[:, c * CB + b:c * CB + b + 1],
                        s[:, b * D:(b + 1) * D],
                        start=mm_state["first"],
                        stop=last and b == CB - 1,
                    )
                    mm_state["first"] = False

            # ---- main loop, software-pipelined by emission order:
            # sub_c | square_{c-1} | matmuls_{c-2}.
            def square(cc):
                s = cslice(cc)
                # the last two squares run on DVE (tensor_mul): by then DVE
                # has no future subs to delay, and it unclogs ACT's backlog
                # so the tail chain ends ~3us earlier.
                if cc >= NCHUNK - 2:
                    nc.vector.tensor_mul(s, s, s)
                else:
                    nc.scalar.activation(s, s, AF.Square)

            issued = 2
            for c in range(NCHUNK):
                if c % 2 == 0 and issued < len(GROUPS):
                    dma_group(issued)
                    issued += 1
                if c == NCHUNK - 1:
                    # emit sq_14 (DVE) before sub_15 so it isn't stuck
                    # behind sub_15's wait for the last DMA group.
                    square(c - 1)
                nc.vector.tensor_sub(cslice(c), cslice(c), lslice(c))
                if c >= 1 and c < NCHUNK - 1:
                    square(c - 1)
                if c >= 2:
                    mm_chunk(c - 2)
            square(NCHUNK - 1)
            mm_chunk(NCHUNK - 2)
            mm_chunk(NCHUNK - 1, last=True)

            # ---- single-scalar output: reduce the PSUM bank on DVE, then
            # one 4-byte DMA.
            res = cpool.tile([1, 1], f32)
            nc.vector.tensor_reduce(res[:], acc[:1, 0:D], axis=AX.X,
                                    op=OP.add)
            nc.sync.dma_start(out.ap(), res[:])

    nc.compile()
    return nc


def _get_program():
    global _program
    if _program is None:
        _program = _build_program()
    return _program


def _make_in_maps(inp, label, ea, attribute, attribute_num):
    import ml_dtypes
    bf16 = ml_dtypes.bfloat16
    inp = np.asarray(inp, dtype=np.float32)
    label = np.asarray(label, dtype=np.float32)
    ea = np.asarray(ea, dtype=np.float32)
    attr_f = np.asarray(attribute, dtype=np.float32)
    anum = np.asarray(attribute_num, dtype=np.float32)
    in_maps = []
    for c in range(N_CORES):
        s = slice(c * BS, (c + 1) * BS)
        dat = np.empty((2, BS, D), dtype=bf16)
        dat[0] = inp[:, s].T.astype(bf16)
        dat[1] = label[:, s].T.astype(bf16)
        # packed weight inputs: [P, 6 | 3*Q (ea j-major) | 6*Q (attr f32)]
        pk = np.empty((P, PK), dtype=np.float32)
        pk[:, 0:6] = anum[None, :]
        pk[:, 6:6 + 3 * Q] = (
            ea[:, s].reshape(3, P, Q).transpose(1, 0, 2).reshape(P, 3 * Q))
        pk[:, 6 + 3 * Q:PK] = (
            attr_f[:, s].reshape(6, P, Q).transpose(1, 0, 2).reshape(P, 6 * Q))
        in_maps.append({"data": dat, "pk": pk})
    return in_maps


def run(inputs, trace=False, trace_cores=None):
    """Run on hardware; returns (result_scalar, BassKernelResults)."""
    try:
        from concourse.bass_utils import run_bass_kernel_spmd
    except ImportError:
        sys.path.insert(0, "/opt/trn_rl_repo")
        from concourse.bass_utils import run_bass_kernel_spmd
    nc = _get_program()
    in_maps = _make_in_maps(**inputs)
    kwargs = {}
    if trace:
        kwargs["trace"] = True
        if trace_cores is not None:
            kwargs["trace_cores"] = trace_cores
    res = run_bass_kernel_spmd(nc, in_maps, core_ids=list(range(N_CORES)), **kwargs)
    total = 0.0
    for r in res.results:
        total += float(r["out"].astype(np.float64).sum())
    value = np.asarray(total / (D * B), dtype=np.float32)
    return value, res


def kernel(**inputs):
    value, _ = run(inputs)
    return value
